# revision 15
# baseline (speedup 1.0000x reference)
"""Trainium2 Bass kernel for nn_DI_net (modulated weight generator).

reference:
    w = leaky_relu(x @ mw_W.T + mw_b)                 # [B, 512]
    weight_out = einsum('ks,bs,sm->bkm', left, w, right)   # [B, 1024, 256]
    weight_out = weight_out.reshape(-1, 256, 1024)         # pure view
    bias_out = leaky_relu(x @ mb_W.T + mb_b)          # [B, 256]
    return (weight_out, bias_out)

Strategy (8 cores, data-parallel over B=256 -> 32 samples/core):
    weight_out[b] = left @ (w[b,:,None] * right)
so the matmul's stationary operand (left.T chunks) is batch-invariant.
Two samples share one matmul via rhs = [R_b0 | R_b1] (N=512, one PSUM
bank).  Matmuls run in float32r (TF32-like) at full PE rate.  left.T is
host-permuted so each sample's output is one contiguous 1 MB DMA.
"""

import numpy as np

import concourse.bacc as bacc
import concourse.bass as bass
import concourse.mybir as mybir
import concourse.tile as tile
from concourse.bass_utils import run_bass_kernel_spmd

N_CORES = 8
B = 256
HOUR = 24
KAUG = HOUR + 1          # ones-row bias trick
S = 512                  # contraction dim (modulation width)
KDIM = 1024              # left rows
M = 256                  # right cols / OUT2
B_LOC = B // N_CORES     # 32
PAIRS = B_LOC // 2       # 16
SC = S // 128            # 4 contraction chunks
KC = KDIM // 128         # 8 output-row chunks

F32 = mybir.dt.float32
F32R = mybir.dt.float32r


def _build_nc() -> bass.Bass:
    nc = bacc.Bacc("TRN2", target_bir_lowering=False, num_swdge_queues=4)

    xt = nc.dram_tensor("xt", [KAUG, B_LOC], F32, kind="ExternalInput")
    mwt = nc.dram_tensor("mwt", [KAUG, S], F32, kind="ExternalInput")
    mbt = nc.dram_tensor("mbt", [KAUG, M], F32, kind="ExternalInput")
    leftT = nc.dram_tensor("leftT", [S, KDIM], F32, kind="ExternalInput")
    right = nc.dram_tensor("right", [S, M], F32, kind="ExternalInput")
    wout = nc.dram_tensor("wout", [B_LOC, KDIM, M], F32, kind="ExternalOutput")
    bout = nc.dram_tensor("bout", [B_LOC, M], F32, kind="ExternalOutput")

    with tile.TileContext(nc) as tc:
        with (
            tc.tile_pool(name="const", bufs=1) as cpool,
            tc.tile_pool(name="lrf32", bufs=2) as lpool,
        ):
            # ---- load params; leftT via SWDGE (Pool) so issues parallelize
            # with the HWDGE (Sync) issues of the small tensors ----
            xt_sb = cpool.tile([KAUG, B_LOC], F32, tag="xt")
            nc.sync.dma_start(xt_sb[:], xt.ap()[:, :])
            mwt_sb = cpool.tile([KAUG, S], F32, tag="mwt")
            nc.sync.dma_start(mwt_sb[:], mwt.ap()[:, :])

            leftT_r = []
            for sc in range(SC):
                lf = lpool.tile([128, KDIM], F32, tag="lf32")
                nc.gpsimd.dma_start(lf[:], leftT.ap()[sc * 128:(sc + 1) * 128, :])
                lr = cpool.tile([128, KDIM], F32R, tag=f"lr{sc}")
                nc.vector.tensor_copy(lr[:], lf[:])
                leftT_r.append(lr)

            right_sb = []
            for sc in range(SC):
                r = cpool.tile([128, M], F32, tag=f"right{sc}")
                nc.sync.dma_start(r[:], right.ap()[sc * 128:(sc + 1) * 128, :])
                right_sb.append(r)

            # ---- prologue: w.T = leaky(mw_W @ x.T) as 4x [128, B_LOC] ----
            wT_sb = []
            with tc.tile_pool(name="propsum", bufs=1, space="PSUM") as ppsum:
                # PE warmup: dummy bf16 matmuls while inputs load, so the HAM
                # clock-gate reaches 8/8 before the real stream starts.
                warm = cpool.tile([128, 512], mybir.dt.bfloat16, tag="warm")
                nc.gpsimd.memset(warm[:], 0)
                wps = ppsum.tile([128, 512], F32, tag="warmps")
                for _ in range(16):
                    nc.tensor.matmul(warm_out := wps[:], warm[:, 0:128], warm[:],
                                     start=True, stop=True)
                # keep the dummies alive: one cheap read
                wkeep = cpool.tile([1, 1], F32, tag="wkeep")
                nc.vector.tensor_copy(wkeep[:], wps[0:1, 0:1])
                for sc in range(SC):
                    wp = ppsum.tile([128, B_LOC], F32, tag=f"wp{sc}")
                    nc.tensor.matmul(
                        wp[:], mwt_sb[:, sc * 128:(sc + 1) * 128], xt_sb[:],
                        start=True, stop=True,
                    )
                    wt = cpool.tile([128, B_LOC], F32, tag=f"wT{sc}")
                    tmp = cpool.tile([128, B_LOC], F32, tag=f"wtmp{sc}")
                    # leaky_relu: max(y, 0.01*y), one PSUM operand per op
                    nc.vector.tensor_scalar_mul(tmp[:], wp[:], 0.01)
                    nc.vector.tensor_max(wt[:], tmp[:], wp[:])
                    wT_sb.append(wt)

                # bias output (off the critical path)
                mbt_sb = cpool.tile([KAUG, M], F32, tag="mbt")
                nc.sync.dma_start(mbt_sb[:], mbt.ap()[:, :])
                bp = ppsum.tile([B_LOC, M], F32, tag="bp")
                nc.tensor.matmul(bp[:], xt_sb[:], mbt_sb[:], start=True, stop=True)
                bsb = cpool.tile([B_LOC, M], F32, tag="bsb")
                btmp = cpool.tile([B_LOC, M], F32, tag="btmp")
                nc.vector.tensor_scalar_mul(btmp[:], bp[:], 0.01)
                nc.vector.tensor_max(bsb[:], btmp[:], bp[:])
                nc.sync.dma_start(bout.ap()[:, :], bsb[:])

            # ---- main loop: 16 sample-pairs ----
            with (
                tc.tile_pool(name="rmod", bufs=3) as rpool,
                tc.tile_pool(name="stage", bufs=5) as spool,
                tc.tile_pool(name="mmpsum", bufs=6, space="PSUM") as mpsum,
            ):
                QTR = 2 * M  # 512 floats/partition = 256 KB quarter-sample
                streams = [nc.sync, nc.gpsimd, nc.scalar]

                for p in range(PAIRS):
                    b0, b1 = 2 * p, 2 * p + 1
                    # modulated right, two samples side by side, rounded to f32r
                    Rt = []
                    for sc in range(SC):
                        R = rpool.tile([128, 2 * M], F32R, tag=f"R{sc}")
                        nc.vector.tensor_scalar_mul(
                            R[:, 0:M], right_sb[sc][:], wT_sb[sc][:, b0:b0 + 1]
                        )
                        nc.vector.tensor_scalar_mul(
                            R[:, M:2 * M], right_sb[sc][:], wT_sb[sc][:, b1:b1 + 1]
                        )
                        Rt.append(R)

                    o0 = spool.tile([128, KC * M], F32, tag="o0")
                    o1 = spool.tile([128, KC * M], F32, tag="o1")
                    # contiguous per-sample views (leftT was host-permuted)
                    v0 = wout.ap()[b0].rearrange("(p f) m -> p (f m)", p=128)
                    v1 = wout.ap()[b1].rearrange("(p f) m -> p (f m)", p=128)
                    for kc in range(KC):
                        pt = mpsum.tile([128, 2 * M], F32, tag="mm")
                        for sc in range(SC):
                            nc.tensor.matmul(
                                pt[:],
                                leftT_r[sc][:, kc * 128:(kc + 1) * 128],
                                Rt[sc][:],
                                start=(sc == 0), stop=(sc == SC - 1),
                            )
                        ca = kc * M
                        cb = (kc + 1) * M
                        if kc % 4 == 3:
                            nc.scalar.copy(o0[:, ca:cb], pt[:, 0:M])
                            nc.vector.tensor_copy(o1[:, ca:cb], pt[:, M:2 * M])
                        elif kc % 4 == 1:
                            nc.vector.tensor_copy(o0[:, ca:cb], pt[:, 0:M])
                            nc.scalar.copy(o1[:, ca:cb], pt[:, M:2 * M])
                        else:
                            nc.vector.tensor_copy(o0[:, ca:cb], pt[:, 0:M])
                            nc.vector.tensor_copy(o1[:, ca:cb], pt[:, M:2 * M])
                        if kc % 2 == 1:
                            # quarter-sample (256 KB) DMAs keep the queues fed
                            q = kc // 2
                            sl = slice(q * QTR, (q + 1) * QTR)
                            streams[(8 * p + 2 * q) % 3].dma_start(v0[:, sl], o0[:, sl])
                            streams[(8 * p + 2 * q + 1) % 3].dma_start(v1[:, sl], o1[:, sl])
    nc.finalize()
    return nc


_NC_CACHE = None
RUN_KWARGS: dict = {}
LAST_RESULT = None


def kernel(x, mw_W, mw_b, mb_W, mb_b, left, right):
    global _NC_CACHE, LAST_RESULT
    x = np.asarray(x, dtype=np.float32)
    mw_W = np.asarray(mw_W, dtype=np.float32)
    mw_b = np.asarray(mw_b, dtype=np.float32)
    mb_W = np.asarray(mb_W, dtype=np.float32)
    mb_b = np.asarray(mb_b, dtype=np.float32)
    left = np.asarray(left, dtype=np.float32)
    right = np.asarray(right, dtype=np.float32)

    # host prep (all tiny except leftT's 2 MB transpose)
    mwt = np.concatenate([mw_W.T, mw_b[None, :]], axis=0)        # [25, 512]
    mbt = np.concatenate([mb_W.T, mb_b[None, :]], axis=0)        # [25, 256]
    # permute left.T columns so column j = kc*128 + p holds k = p*8 + kc:
    # the per-sample output DMA then lands contiguous in [1024, 256] k-major.
    leftT = np.ascontiguousarray(
        left.T.reshape(S, 128, KC).transpose(0, 2, 1).reshape(S, KDIM)
    )
    right_c = np.ascontiguousarray(right)

    if _NC_CACHE is None:
        _NC_CACHE = _build_nc()
    nc = _NC_CACHE

    in_maps = []
    for c in range(N_CORES):
        xs = x[c * B_LOC:(c + 1) * B_LOC]                         # [32, 24]
        xt = np.concatenate(
            [np.ascontiguousarray(xs.T), np.ones((1, B_LOC), np.float32)], axis=0
        )                                                         # [25, 32]
        in_maps.append({
            "xt": xt, "mwt": mwt, "mbt": mbt,
            "leftT": leftT, "right": right_c,
        })

    res = run_bass_kernel_spmd(nc, in_maps, core_ids=list(range(N_CORES)),
                               **RUN_KWARGS)
    LAST_RESULT = res
    wout = np.concatenate([r["wout"] for r in res.results], axis=0)  # [256,1024,256]
    bout = np.concatenate([r["bout"] for r in res.results], axis=0)  # [256,256]
    return (wout.reshape(-1, M, KDIM), bout)


# revision 16
# speedup vs baseline: 1.0392x; 1.0392x over previous
"""Trainium2 Bass kernel for nn_DI_net (modulated weight generator).

reference:
    w = leaky_relu(x @ mw_W.T + mw_b)                 # [B, 512]
    weight_out = einsum('ks,bs,sm->bkm', left, w, right)   # [B, 1024, 256]
    weight_out = weight_out.reshape(-1, 256, 1024)         # pure view
    bias_out = leaky_relu(x @ mb_W.T + mb_b)          # [B, 256]
    return (weight_out, bias_out)

Strategy (8 cores, data-parallel over B=256 -> 32 samples/core):
    weight_out[b] = left @ (w[b,:,None] * right)
so the matmul's stationary operand (left.T chunks) is batch-invariant.
Two samples share one matmul via rhs = [R_b0 | R_b1] (N=512, one PSUM
bank).  Matmuls run in float32r (TF32-like) at full PE rate.  left.T is
host-permuted so each sample's output is one contiguous 1 MB DMA.
"""

import numpy as np

import concourse.bacc as bacc
import concourse.bass as bass
import concourse.mybir as mybir
import concourse.tile as tile
from concourse.bass_utils import run_bass_kernel_spmd

N_CORES = 8
B = 256
HOUR = 24
KAUG = HOUR + 1          # ones-row bias trick
S = 512                  # contraction dim (modulation width)
KDIM = 1024              # left rows
M = 256                  # right cols / OUT2
B_LOC = B // N_CORES     # 32
PAIRS = B_LOC // 2       # 16
SC = S // 128            # 4 contraction chunks
KC = KDIM // 128         # 8 output-row chunks

F32 = mybir.dt.float32
F32R = mybir.dt.float32r


def _build_nc() -> bass.Bass:
    nc = bacc.Bacc("TRN2", target_bir_lowering=False, num_swdge_queues=4)

    xt = nc.dram_tensor("xt", [KAUG, B_LOC], F32, kind="ExternalInput")
    mwt = nc.dram_tensor("mwt", [KAUG, S], F32, kind="ExternalInput")
    mbt = nc.dram_tensor("mbt", [KAUG, M], F32, kind="ExternalInput")
    leftT = nc.dram_tensor("leftT", [S, KDIM], F32, kind="ExternalInput")
    right = nc.dram_tensor("right", [S, M], F32, kind="ExternalInput")
    wout = nc.dram_tensor("wout", [B_LOC, KDIM, M], F32, kind="ExternalOutput")
    bout = nc.dram_tensor("bout", [B_LOC, M], F32, kind="ExternalOutput")

    with tile.TileContext(nc) as tc:
        with (
            tc.tile_pool(name="const", bufs=1) as cpool,
            tc.tile_pool(name="lrf32", bufs=2) as lpool,
        ):
            # ---- load params; leftT via SWDGE (Pool) so issues parallelize
            # with the HWDGE (Sync) issues of the small tensors ----
            xt_sb = cpool.tile([KAUG, B_LOC], F32, tag="xt")
            nc.sync.dma_start(xt_sb[:], xt.ap()[:, :])
            mwt_sb = cpool.tile([KAUG, S], F32, tag="mwt")
            nc.sync.dma_start(mwt_sb[:], mwt.ap()[:, :])

            leftT_r = []
            for sc in range(SC):
                lf = lpool.tile([128, KDIM], F32, tag="lf32")
                nc.gpsimd.dma_start(lf[:], leftT.ap()[sc * 128:(sc + 1) * 128, :])
                lr = cpool.tile([128, KDIM], F32R, tag=f"lr{sc}")
                nc.vector.tensor_copy(lr[:], lf[:])
                leftT_r.append(lr)

            right_sb = []
            for sc in range(SC):
                r = cpool.tile([128, M], F32, tag=f"right{sc}")
                nc.sync.dma_start(r[:], right.ap()[sc * 128:(sc + 1) * 128, :])
                right_sb.append(r)

            # ---- prologue: w.T = leaky(mw_W @ x.T) as 4x [128, B_LOC] ----
            wT_sb = []
            with tc.tile_pool(name="propsum", bufs=1, space="PSUM") as ppsum:
                for sc in range(SC):
                    wp = ppsum.tile([128, B_LOC], F32, tag=f"wp{sc}")
                    nc.tensor.matmul(
                        wp[:], mwt_sb[:, sc * 128:(sc + 1) * 128], xt_sb[:],
                        start=True, stop=True,
                    )
                    wt = cpool.tile([128, B_LOC], F32, tag=f"wT{sc}")
                    tmp = cpool.tile([128, B_LOC], F32, tag=f"wtmp{sc}")
                    # leaky_relu: max(y, 0.01*y), one PSUM operand per op
                    nc.vector.tensor_scalar_mul(tmp[:], wp[:], 0.01)
                    nc.vector.tensor_max(wt[:], tmp[:], wp[:])
                    wT_sb.append(wt)

                # bias output (off the critical path)
                mbt_sb = cpool.tile([KAUG, M], F32, tag="mbt")
                nc.sync.dma_start(mbt_sb[:], mbt.ap()[:, :])
                bp = ppsum.tile([B_LOC, M], F32, tag="bp")
                nc.tensor.matmul(bp[:], xt_sb[:], mbt_sb[:], start=True, stop=True)
                bsb = cpool.tile([B_LOC, M], F32, tag="bsb")
                btmp = cpool.tile([B_LOC, M], F32, tag="btmp")
                nc.vector.tensor_scalar_mul(btmp[:], bp[:], 0.01)
                nc.vector.tensor_max(bsb[:], btmp[:], bp[:])
                nc.sync.dma_start(bout.ap()[:, :], bsb[:])

            # ---- main loop: 16 sample-pairs ----
            with (
                tc.tile_pool(name="rmod", bufs=3) as rpool,
                tc.tile_pool(name="stage", bufs=5) as spool,
                tc.tile_pool(name="mmpsum", bufs=6, space="PSUM") as mpsum,
            ):
                QTR = 2 * M  # 512 floats/partition = 256 KB quarter-sample
                streams = [nc.sync, nc.gpsimd, nc.scalar]

                for p in range(PAIRS):
                    b0, b1 = 2 * p, 2 * p + 1
                    # modulated right, two samples side by side, rounded to f32r
                    Rt = []
                    for sc in range(SC):
                        R = rpool.tile([128, 2 * M], F32R, tag=f"R{sc}")
                        nc.vector.tensor_scalar_mul(
                            R[:, 0:M], right_sb[sc][:], wT_sb[sc][:, b0:b0 + 1]
                        )
                        nc.vector.tensor_scalar_mul(
                            R[:, M:2 * M], right_sb[sc][:], wT_sb[sc][:, b1:b1 + 1]
                        )
                        Rt.append(R)

                    o0 = spool.tile([128, KC * M], F32, tag="o0")
                    o1 = spool.tile([128, KC * M], F32, tag="o1")
                    # contiguous per-sample views (leftT was host-permuted)
                    v0 = wout.ap()[b0].rearrange("(p f) m -> p (f m)", p=128)
                    v1 = wout.ap()[b1].rearrange("(p f) m -> p (f m)", p=128)
                    for kc in range(KC):
                        pt = mpsum.tile([128, 2 * M], F32, tag="mm")
                        for sc in range(SC):
                            nc.tensor.matmul(
                                pt[:],
                                leftT_r[sc][:, kc * 128:(kc + 1) * 128],
                                Rt[sc][:],
                                start=(sc == 0), stop=(sc == SC - 1),
                            )
                        ca = kc * M
                        cb = (kc + 1) * M
                        if kc % 4 == 3:
                            nc.scalar.copy(o0[:, ca:cb], pt[:, 0:M])
                            nc.vector.tensor_copy(o1[:, ca:cb], pt[:, M:2 * M])
                        elif kc % 4 == 1:
                            nc.vector.tensor_copy(o0[:, ca:cb], pt[:, 0:M])
                            nc.scalar.copy(o1[:, ca:cb], pt[:, M:2 * M])
                        else:
                            nc.vector.tensor_copy(o0[:, ca:cb], pt[:, 0:M])
                            nc.vector.tensor_copy(o1[:, ca:cb], pt[:, M:2 * M])
                        if kc % 2 == 1:
                            # quarter-sample (256 KB) DMAs keep the queues fed
                            q = kc // 2
                            sl = slice(q * QTR, (q + 1) * QTR)
                            streams[(8 * p + 2 * q) % 3].dma_start(v0[:, sl], o0[:, sl])
                            streams[(8 * p + 2 * q + 1) % 3].dma_start(v1[:, sl], o1[:, sl])
    nc.finalize()
    return nc


_NC_CACHE = None
RUN_KWARGS: dict = {}
LAST_RESULT = None


def kernel(x, mw_W, mw_b, mb_W, mb_b, left, right):
    global _NC_CACHE, LAST_RESULT
    x = np.asarray(x, dtype=np.float32)
    mw_W = np.asarray(mw_W, dtype=np.float32)
    mw_b = np.asarray(mw_b, dtype=np.float32)
    mb_W = np.asarray(mb_W, dtype=np.float32)
    mb_b = np.asarray(mb_b, dtype=np.float32)
    left = np.asarray(left, dtype=np.float32)
    right = np.asarray(right, dtype=np.float32)

    # host prep (all tiny except leftT's 2 MB transpose)
    mwt = np.concatenate([mw_W.T, mw_b[None, :]], axis=0)        # [25, 512]
    mbt = np.concatenate([mb_W.T, mb_b[None, :]], axis=0)        # [25, 256]
    # permute left.T columns so column j = kc*128 + p holds k = p*8 + kc:
    # the per-sample output DMA then lands contiguous in [1024, 256] k-major.
    leftT = np.ascontiguousarray(
        left.T.reshape(S, 128, KC).transpose(0, 2, 1).reshape(S, KDIM)
    )
    right_c = np.ascontiguousarray(right)

    if _NC_CACHE is None:
        _NC_CACHE = _build_nc()
    nc = _NC_CACHE

    in_maps = []
    for c in range(N_CORES):
        xs = x[c * B_LOC:(c + 1) * B_LOC]                         # [32, 24]
        xt = np.concatenate(
            [np.ascontiguousarray(xs.T), np.ones((1, B_LOC), np.float32)], axis=0
        )                                                         # [25, 32]
        in_maps.append({
            "xt": xt, "mwt": mwt, "mbt": mbt,
            "leftT": leftT, "right": right_c,
        })

    res = run_bass_kernel_spmd(nc, in_maps, core_ids=list(range(N_CORES)),
                               **RUN_KWARGS)
    LAST_RESULT = res
    wout = np.concatenate([r["wout"] for r in res.results], axis=0)  # [256,1024,256]
    bout = np.concatenate([r["bout"] for r in res.results], axis=0)  # [256,256]
    return (wout.reshape(-1, M, KDIM), bout)


# revision 18
# speedup vs baseline: 1.0788x; 1.0382x over previous
"""Trainium2 Bass kernel for nn_DI_net (modulated weight generator).

reference:
    w = leaky_relu(x @ mw_W.T + mw_b)                 # [B, 512]
    weight_out = einsum('ks,bs,sm->bkm', left, w, right)   # [B, 1024, 256]
    weight_out = weight_out.reshape(-1, 256, 1024)         # pure view
    bias_out = leaky_relu(x @ mb_W.T + mb_b)          # [B, 256]
    return (weight_out, bias_out)

Strategy (8 cores, data-parallel over B=256 -> 32 samples/core):
    weight_out[b] = left @ (w[b,:,None] * right)
so the matmul's stationary operand (left.T chunks) is batch-invariant.
Two samples share one matmul via rhs = [R_b0 | R_b1] (N=512, one PSUM
bank).  Matmuls run in float32r (TF32-like) at full PE rate.  left.T is
host-permuted so each sample's output is one contiguous 1 MB DMA.
"""

import numpy as np

import concourse.bacc as bacc
import concourse.bass as bass
import concourse.mybir as mybir
import concourse.tile as tile
from concourse.bass_utils import run_bass_kernel_spmd

N_CORES = 8
B = 256
HOUR = 24
KAUG = HOUR + 1          # ones-row bias trick
S = 512                  # contraction dim (modulation width)
KDIM = 1024              # left rows
M = 256                  # right cols / OUT2
B_LOC = B // N_CORES     # 32
PAIRS = B_LOC // 2       # 16
SC = S // 128            # 4 contraction chunks
KC = KDIM // 128         # 8 output-row chunks

F32 = mybir.dt.float32
F32R = mybir.dt.float32r


def _build_nc() -> bass.Bass:
    nc = bacc.Bacc("TRN2", target_bir_lowering=False, num_swdge_queues=4)

    xt = nc.dram_tensor("xt", [KAUG, B_LOC], F32, kind="ExternalInput")
    mwt = nc.dram_tensor("mwt", [KAUG, S], F32, kind="ExternalInput")
    mbt = nc.dram_tensor("mbt", [KAUG, M], F32, kind="ExternalInput")
    leftT = nc.dram_tensor("leftT", [S, KDIM], F32, kind="ExternalInput")
    right = nc.dram_tensor("right", [S, M], F32, kind="ExternalInput")
    wout = nc.dram_tensor("wout", [B_LOC, KDIM, M], F32, kind="ExternalOutput")
    bout = nc.dram_tensor("bout", [B_LOC, M], F32, kind="ExternalOutput")

    with tile.TileContext(nc) as tc:
        with (
            tc.tile_pool(name="const", bufs=1) as cpool,
            tc.tile_pool(name="lrf32", bufs=2) as lpool,
        ):
            # ---- load params; leftT via SWDGE (Pool) so issues parallelize
            # with the HWDGE (Sync) issues of the small tensors ----
            xt_sb = cpool.tile([KAUG, B_LOC], F32, tag="xt")
            nc.sync.dma_start(xt_sb[:], xt.ap()[:, :])
            mwt_sb = cpool.tile([KAUG, S], F32, tag="mwt")
            nc.sync.dma_start(mwt_sb[:], mwt.ap()[:, :])

            leftT_r = []
            for sc in range(SC):
                lf = lpool.tile([128, KDIM], F32, tag="lf32")
                nc.gpsimd.dma_start(lf[:], leftT.ap()[sc * 128:(sc + 1) * 128, :])
                lr = cpool.tile([128, KDIM], mybir.dt.bfloat16, tag=f"lr{sc}")
                nc.vector.tensor_copy(lr[:], lf[:])
                leftT_r.append(lr)

            right_sb = []
            for sc in range(SC):
                r = cpool.tile([128, M], F32, tag=f"right{sc}")
                nc.sync.dma_start(r[:], right.ap()[sc * 128:(sc + 1) * 128, :])
                right_sb.append(r)

            # ---- prologue: w.T = leaky(mw_W @ x.T) as 4x [128, B_LOC] ----
            wT_sb = []
            with tc.tile_pool(name="propsum", bufs=1, space="PSUM") as ppsum:
                for sc in range(SC):
                    wp = ppsum.tile([128, B_LOC], F32, tag=f"wp{sc}")
                    nc.tensor.matmul(
                        wp[:], mwt_sb[:, sc * 128:(sc + 1) * 128], xt_sb[:],
                        start=True, stop=True,
                    )
                    wt = cpool.tile([128, B_LOC], F32, tag=f"wT{sc}")
                    tmp = cpool.tile([128, B_LOC], F32, tag=f"wtmp{sc}")
                    # leaky_relu: max(y, 0.01*y), one PSUM operand per op
                    nc.vector.tensor_scalar_mul(tmp[:], wp[:], 0.01)
                    nc.vector.tensor_max(wt[:], tmp[:], wp[:])
                    wT_sb.append(wt)

                # bias output (off the critical path)
                mbt_sb = cpool.tile([KAUG, M], F32, tag="mbt")
                nc.sync.dma_start(mbt_sb[:], mbt.ap()[:, :])
                bp = ppsum.tile([B_LOC, M], F32, tag="bp")
                nc.tensor.matmul(bp[:], xt_sb[:], mbt_sb[:], start=True, stop=True)
                bsb = cpool.tile([B_LOC, M], F32, tag="bsb")
                btmp = cpool.tile([B_LOC, M], F32, tag="btmp")
                nc.vector.tensor_scalar_mul(btmp[:], bp[:], 0.01)
                nc.vector.tensor_max(bsb[:], btmp[:], bp[:])
                nc.sync.dma_start(bout.ap()[:, :], bsb[:])

            # ---- main loop: 16 sample-pairs ----
            with (
                tc.tile_pool(name="rmod", bufs=3) as rpool,
                tc.tile_pool(name="stage", bufs=5) as spool,
                tc.tile_pool(name="mmpsum", bufs=6, space="PSUM") as mpsum,
            ):
                QTR = 2 * M  # 512 floats/partition = 256 KB quarter-sample
                streams = [nc.sync, nc.gpsimd, nc.scalar]

                for p in range(PAIRS):
                    b0, b1 = 2 * p, 2 * p + 1
                    # modulated right, two samples side by side, rounded to f32r
                    Rt = []
                    for sc in range(SC):
                        R = rpool.tile([128, 2 * M], mybir.dt.bfloat16, tag=f"R{sc}")
                        nc.vector.tensor_scalar_mul(
                            R[:, 0:M], right_sb[sc][:], wT_sb[sc][:, b0:b0 + 1]
                        )
                        nc.vector.tensor_scalar_mul(
                            R[:, M:2 * M], right_sb[sc][:], wT_sb[sc][:, b1:b1 + 1]
                        )
                        Rt.append(R)

                    o0 = spool.tile([128, KC * M], F32, tag="o0")
                    o1 = spool.tile([128, KC * M], F32, tag="o1")
                    # contiguous per-sample views (leftT was host-permuted)
                    v0 = wout.ap()[b0].rearrange("(p f) m -> p (f m)", p=128)
                    v1 = wout.ap()[b1].rearrange("(p f) m -> p (f m)", p=128)
                    for kc in range(KC):
                        pt = mpsum.tile([128, 2 * M], F32, tag="mm")
                        for sc in range(SC):
                            nc.tensor.matmul(
                                pt[:],
                                leftT_r[sc][:, kc * 128:(kc + 1) * 128],
                                Rt[sc][:],
                                start=(sc == 0), stop=(sc == SC - 1),
                            )
                        ca = kc * M
                        cb = (kc + 1) * M
                        if kc % 4 == 3:
                            nc.scalar.copy(o0[:, ca:cb], pt[:, 0:M])
                            nc.vector.tensor_copy(o1[:, ca:cb], pt[:, M:2 * M])
                        elif kc % 4 == 1:
                            nc.vector.tensor_copy(o0[:, ca:cb], pt[:, 0:M])
                            nc.scalar.copy(o1[:, ca:cb], pt[:, M:2 * M])
                        else:
                            nc.vector.tensor_copy(o0[:, ca:cb], pt[:, 0:M])
                            nc.vector.tensor_copy(o1[:, ca:cb], pt[:, M:2 * M])
                        if kc % 2 == 1:
                            # quarter-sample (256 KB) DMAs keep the queues fed
                            q = kc // 2
                            sl = slice(q * QTR, (q + 1) * QTR)
                            streams[(8 * p + 2 * q) % 3].dma_start(v0[:, sl], o0[:, sl])
                            streams[(8 * p + 2 * q + 1) % 3].dma_start(v1[:, sl], o1[:, sl])
    nc.finalize()
    return nc


_NC_CACHE = None
RUN_KWARGS: dict = {}
LAST_RESULT = None


def kernel(x, mw_W, mw_b, mb_W, mb_b, left, right):
    global _NC_CACHE, LAST_RESULT
    x = np.asarray(x, dtype=np.float32)
    mw_W = np.asarray(mw_W, dtype=np.float32)
    mw_b = np.asarray(mw_b, dtype=np.float32)
    mb_W = np.asarray(mb_W, dtype=np.float32)
    mb_b = np.asarray(mb_b, dtype=np.float32)
    left = np.asarray(left, dtype=np.float32)
    right = np.asarray(right, dtype=np.float32)

    # host prep (all tiny except leftT's 2 MB transpose)
    mwt = np.concatenate([mw_W.T, mw_b[None, :]], axis=0)        # [25, 512]
    mbt = np.concatenate([mb_W.T, mb_b[None, :]], axis=0)        # [25, 256]
    # permute left.T columns so column j = kc*128 + p holds k = p*8 + kc:
    # the per-sample output DMA then lands contiguous in [1024, 256] k-major.
    leftT = np.ascontiguousarray(
        left.T.reshape(S, 128, KC).transpose(0, 2, 1).reshape(S, KDIM)
    )
    right_c = np.ascontiguousarray(right)

    if _NC_CACHE is None:
        _NC_CACHE = _build_nc()
    nc = _NC_CACHE

    in_maps = []
    for c in range(N_CORES):
        xs = x[c * B_LOC:(c + 1) * B_LOC]                         # [32, 24]
        xt = np.concatenate(
            [np.ascontiguousarray(xs.T), np.ones((1, B_LOC), np.float32)], axis=0
        )                                                         # [25, 32]
        in_maps.append({
            "xt": xt, "mwt": mwt, "mbt": mbt,
            "leftT": leftT, "right": right_c,
        })

    res = run_bass_kernel_spmd(nc, in_maps, core_ids=list(range(N_CORES)),
                               **RUN_KWARGS)
    LAST_RESULT = res
    wout = np.concatenate([r["wout"] for r in res.results], axis=0)  # [256,1024,256]
    bout = np.concatenate([r["bout"] for r in res.results], axis=0)  # [256,256]
    return (wout.reshape(-1, M, KDIM), bout)
